# revision 1
# baseline (speedup 1.0000x reference)
"""Multi-head attention (qkv proj + softmax attention + out proj) on 8 trn2 cores.

Sharding: zero-collective. Core c handles batch b=c//2 and query-half h=c%2
(1024 queries). The host passes x[b] ROTATED by h*1024 tokens so that each
core's queries are always local tokens 0..1023 while K/V still cover all 2048
tokens (key order is irrelevant to softmax). Host concatenates the 8 output
slabs [1024, 1024] into [4, 2048, 1024].

Per-core kernel (all on one NeuronCore, Tile-scheduled):
  1. PE-transpose x and the weights to contraction-major layout (fp32 in,
     bf16 out via the DVE PSUM-evacuation copy).
  2. QKV projection in bf16: QT/KT stored feature-major [d, t] so each
     128-row tile holds a PAIR of heads (2 x 64 d-rows); V stored token-major
     with interleaved ones columns (fused softmax-sum rows come out of the
     AV matmul for free).
  3. Attention per head-pair: QK^T into PSUM, softmax WITHOUT
     max-subtraction (scores are ~N(0,1); fp32 exp is safe), exp on ScalarE
     straight out of PSUM, AV+sums fused, fast approximate reciprocal on DVE.
  4. Out-projection in bf16 + bias, fp32 result.
"""

import numpy as np

B, N, C = 4, 2048, 1024
H, D = 16, 64
P = 128
CG = C // P            # 8 contraction groups
TG = N // P            # 16 key-token chunks
TQ = N // 2            # 1024 queries per core
QB = 512               # query block (psum bank)
NB = TQ // QB          # 2
NPAIR = H // 2         # 8 head pairs
SCALE = 1.0 / np.sqrt(D).astype(np.float32)
DEBUG_DUMPS = False

_CACHE = {}


def _build():
    import concourse.tile as tile
    from concourse import bacc, mybir

    f32 = mybir.dt.float32
    nc = bacc.Bacc(
        "TRN2", target_bir_lowering=False, debug=False, num_devices=8
    )
    x_h = nc.dram_tensor("x", [N, C], f32, kind="ExternalInput").ap()
    wqkv_h = nc.dram_tensor("w_qkv", [3 * C, C], f32, kind="ExternalInput").ap()
    wout_h = nc.dram_tensor("w_out", [C, C], f32, kind="ExternalInput").ap()
    bout_h = nc.dram_tensor("b_out", [C], f32, kind="ExternalInput").ap()
    y_h = nc.dram_tensor("y", [TQ, C], f32, kind="ExternalOutput").ap()

    with tile.TileContext(nc) as tc:
        _emit(tc, x_h, wqkv_h, wout_h, bout_h, y_h)
    nc.compile()
    return nc


def _emit(tc, x_h, wqkv_h, wout_h, bout_h, y_h):
    from contextlib import ExitStack

    from concourse import mybir
    from concourse.masks import make_identity

    f32 = mybir.dt.float32
    bf16 = mybir.dt.bfloat16
    AF = mybir.ActivationFunctionType
    nc = tc.nc

    with ExitStack() as ctx:
        # ---------------- pools ----------------
        const = ctx.enter_context(tc.tile_pool(name="const", bufs=1))
        big = ctx.enter_context(tc.tile_pool(name="big", bufs=1))
        land = ctx.enter_context(tc.tile_pool(name="land", bufs=3))
        cstp = ctx.enter_context(tc.tile_pool(name="cstp", bufs=6))
        ktp = ctx.enter_context(tc.tile_pool(name="ktp", bufs=2))
        qtp = ctx.enter_context(tc.tile_pool(name="qtp", bufs=2))
        wtq = ctx.enter_context(tc.tile_pool(name="wtq", bufs=2))
        wtv = ctx.enter_context(tc.tile_pool(name="wtv", bufs=2))
        ptp = ctx.enter_context(tc.tile_pool(name="ptp", bufs=15))
        recp = ctx.enter_context(tc.tile_pool(name="recp", bufs=2))
        outp = ctx.enter_context(tc.tile_pool(name="outp", bufs=2))
        ps_pool = ctx.enter_context(
            tc.tile_pool(name="ps_pool", bufs=2, space="PSUM")
        )
        st_ps = ctx.enter_context(
            tc.tile_pool(name="st_ps", bufs=2, space="PSUM")
        )
        at_ps = ctx.enter_context(
            tc.tile_pool(name="at_ps", bufs=2, space="PSUM")
        )

        # ---------------- constants ----------------
        ident = const.tile([P, P], bf16)
        make_identity(nc, ident)
        onesrow = const.tile([1, P], f32)
        nc.gpsimd.memset(onesrow, 1.0)

        # ---------------- persistent tensors ----------------
        # V layout per pair p: cols [192p,192p+64)=V_h2p, [192p+64,+128)=ones,
        # [192p+128,+192)=V_h2p+1. The AV stationary operand for head A is
        # cols [192p,192p+128) = [V_A | ones] (output rows 0:64 = attention
        # out, rows 64:128 = softmax sums, replicated); for head B it is cols
        # [192p+64,192p+192) = [ones | V_B] (sums on rows 0:64, attention out
        # on rows 64:128). The ones block is shared between the two heads.
        xT = [big.tile([P, N], bf16, name=f"xT{g}") for g in range(CG)]
        V = [big.tile([P, 192 * NPAIR], bf16, name=f"V{i}") for i in range(TG)]
        aoT = [big.tile([P, TQ], bf16, name=f"aoT{g}") for g in range(CG)]
        for i in range(TG):
            v3i = V[i].rearrange("p (q e) -> p q e", e=64)
            nc.vector.memset(v3i[:, 1 : 3 * NPAIR : 3], 1.0)

        # -------- load x (fp32), cast bf16, PE-transpose into xT --------
        def load_cast(src_ap, name, scale=None):
            t = land.tile([P, C], f32, tag="land", name=f"ld{name}")
            nc.sync.dma_start(t, src_ap)
            b = cstp.tile([P, C], bf16, tag="cst", name=f"cs{name}")
            if scale is None:
                nc.vector.tensor_copy(b, t)
            else:
                nc.vector.tensor_scalar_mul(b, t, scale)
            return b

        xb16 = []
        for i in range(TG):
            xb16.append(load_cast(x_h[i * P : (i + 1) * P, :], f"x{i}"))

        # 4 [128,128] bf16 PE transposes into one psum bank, one copy.
        for i4 in range(0, TG, 4):
            for g in range(CG):
                psf = ps_pool.tile([P, QB], f32, tag="ps")
                ps = psf[:, :].bitcast(bf16)[:, 0 : 4 * P]
                for k in range(4):
                    nc.tensor.transpose(
                        ps[:, k * P : (k + 1) * P],
                        xb16[i4 + k][:, g * P : (g + 1) * P],
                        ident,
                    )
                nc.vector.tensor_copy(xT[g][:, i4 * P : (i4 + 4) * P], ps)

        def w_chunk_T(src_h, row0, pool, tag):
            """Load+transpose 4 consecutive 128-row chunks of a [*, C] fp32
            weight into a [P, CG, 512] bf16 tile (contraction-major)."""
            wt = pool.tile([P, CG, 4 * P], bf16, tag=tag)
            wn = []
            for jj in range(4):
                wn.append(
                    load_cast(
                        src_h[(row0 + jj) * P : (row0 + jj + 1) * P, :],
                        f"w{row0}_{jj}",
                    )
                )
            for g in range(CG):
                psf = ps_pool.tile([P, QB], f32, tag="ps")
                ps = psf[:, :].bitcast(bf16)[:, 0 : 4 * P]
                for jj in range(4):
                    nc.tensor.transpose(
                        ps[:, jj * P : (jj + 1) * P],
                        wn[jj][:, g * P : (g + 1) * P],
                        ident,
                    )
                nc.vector.tensor_copy(wt[:, g, :], ps)
            return wt

        # ---------------- V projection ----------------
        for fh in range(2):
            wvT = w_chunk_T(wqkv_h, 16 + 4 * fh, wtv, "wtv")
            for i in range(TG):
                ps = ps_pool.tile([P, QB], f32, tag="ps")
                for g in range(CG):
                    nc.tensor.matmul(
                        ps,
                        xT[g][:, i * P : (i + 1) * P],
                        wvT[:, g, :],
                        start=(g == 0),
                        stop=(g == CG - 1),
                    )
                # psum cols = v features [512*fh, 512*(fh+1)) = heads
                # 8fh..8fh+7. Scatter per-head 64-col blocks into the
                # interleaved V layout: head h -> col 192*(h//2)+128*(h%2).
                ps3 = ps.rearrange("p (k e) -> p k e", e=64)
                v3 = V[i].rearrange("p (q e) -> p q e", e=64)
                b0 = 12 * fh
                nc.vector.tensor_copy(v3[:, b0 : b0 + 12 : 3], ps3[:, 0::2])
                nc.vector.tensor_copy(
                    v3[:, b0 + 2 : b0 + 12 : 3], ps3[:, 1::2]
                )

        # ---------------- per-pair: K proj, Q proj, attention ----------------
        for p in range(NPAIR):
            # K chunk p: w_qkv rows [C + p*128, C + (p+1)*128)
            KT = ktp.tile([P, N], bf16, tag="KT")
            wt = wtq.tile([P, CG, P], bf16, tag="wtq")
            wn = load_cast(wqkv_h[(8 + p) * P : (9 + p) * P, :], f"wk{p}")
            for g4 in range(0, CG, 4):
                psf = ps_pool.tile([P, QB], f32, tag="ps")
                ps = psf[:, :].bitcast(bf16)[:, 0 : 4 * P]
                for k in range(4):
                    nc.tensor.transpose(
                        ps[:, k * P : (k + 1) * P],
                        wn[:, (g4 + k) * P : (g4 + k + 1) * P],
                        ident,
                    )
                nc.vector.tensor_copy(wt[:, g4 : g4 + 4, :], ps)
            for tbp in range(0, 4, 2):
                pss = [
                    ps_pool.tile([P, QB], f32, tag="ps", name=f"pjk{k}")
                    for k in range(2)
                ]
                for g in range(CG):
                    for k in range(2):
                        nc.tensor.matmul(
                            pss[k],
                            wt[:, g, :],
                            xT[g][:, (tbp + k) * QB : (tbp + k + 1) * QB],
                            start=(g == 0),
                            stop=(g == CG - 1),
                        )
                for k in range(2):
                    nc.vector.tensor_copy(
                        KT[:, (tbp + k) * QB : (tbp + k + 1) * QB], pss[k]
                    )

            # Q chunk p: w_qkv rows [p*128, (p+1)*128); queries are tokens
            # 0..TQ-1 of the (rotated) local x. Scale 1/sqrt(D) folded into
            # the psum evacuation copy.
            QT = qtp.tile([P, TQ], bf16, tag="QT")
            wt = wtq.tile([P, CG, P], bf16, tag="wtq")
            wn = load_cast(
                wqkv_h[p * P : (p + 1) * P, :], f"wq{p}", scale=float(SCALE)
            )
            for g4 in range(0, CG, 4):
                psf = ps_pool.tile([P, QB], f32, tag="ps")
                ps = psf[:, :].bitcast(bf16)[:, 0 : 4 * P]
                for k in range(4):
                    nc.tensor.transpose(
                        ps[:, k * P : (k + 1) * P],
                        wn[:, (g4 + k) * P : (g4 + k + 1) * P],
                        ident,
                    )
                nc.vector.tensor_copy(wt[:, g4 : g4 + 4, :], ps)
            pss = [
                ps_pool.tile([P, QB], f32, tag="ps", name=f"pjq{k}")
                for k in range(2)
            ]
            for g in range(CG):
                for k in range(2):
                    nc.tensor.matmul(
                        pss[k],
                        wt[:, g, :],
                        xT[g][:, k * QB : (k + 1) * QB],
                        start=(g == 0),
                        stop=(g == CG - 1),
                    )
            for k in range(2):
                nc.vector.tensor_copy(QT[:, k * QB : (k + 1) * QB], pss[k])

            # ---- attention for head pair p ----
            for tb in range(NB):
                qa = QT[0:64, tb * QB : (tb + 1) * QB]
                qb = QT[64:128, tb * QB : (tb + 1) * QB]
                pts = []
                for jj in range(TG // 2):
                    sta = st_ps.tile([P, 2 * QB], f32, tag="st", name="sta")
                    stb = st_ps.tile([P, 2 * QB], f32, tag="st", name="stb")
                    for k in range(2):
                        j = 2 * jj + k
                        nc.tensor.matmul(
                            sta[:, k * QB : (k + 1) * QB],
                            KT[0:64, j * P : (j + 1) * P],
                            qa,
                        )
                        nc.tensor.matmul(
                            stb[:, k * QB : (k + 1) * QB],
                            KT[64:128, j * P : (j + 1) * P],
                            qb,
                        )
                    pta = ptp.tile([P, 2 * QB], bf16, tag="pt", name="pta")
                    ptb = ptp.tile([P, 2 * QB], bf16, tag="pt", name="ptb")
                    nc.scalar.activation(pta, sta, AF.Exp)
                    nc.scalar.activation(ptb, stb, AF.Exp)
                    pts.append((pta, ptb))

                # AV with fused softmax-sums (see V layout comment).
                ata = at_ps.tile([P, QB], f32, tag="at", name="ata")
                atb = at_ps.tile([P, QB], f32, tag="at", name="atb")
                for j in range(TG):
                    jj, k = divmod(j, 2)
                    pta, ptb = pts[jj]
                    first, last = (j == 0), (j == TG - 1)
                    nc.tensor.matmul(
                        ata,
                        V[j][:, 192 * p : 192 * p + 128],
                        pta[:, k * QB : (k + 1) * QB],
                        start=first,
                        stop=last,
                    )
                    nc.tensor.matmul(
                        atb,
                        V[j][:, 192 * p + 64 : 192 * p + 192],
                        ptb[:, k * QB : (k + 1) * QB],
                        start=first,
                        stop=last,
                    )

                # normalize: out = at * (1/sum). reciprocal_approx_fast
                # (custom-DVE) requires base-partition-0 APs, so stage sumsA
                # down to a base-0 tile; mixed PSUM+SBUF operands may use
                # different base partitions, so the muls read the reciprocal
                # tiles directly.
                combA = recp.tile([64, QB], f32, tag="combA", bufs=1)
                nc.vector.tensor_copy(combA, ata[64:128, :])
                rtA = recp.tile([64, QB], f32, tag="rtA", bufs=1)
                nc.vector.reciprocal_approx_fast(rtA, combA)
                rtB = recp.tile([64, QB], f32, tag="rtB", bufs=1)
                nc.vector.reciprocal_approx_fast(rtB, atb[0:64, :])
                ao = aoT[p][:, tb * QB : (tb + 1) * QB]
                nc.vector.tensor_mul(ao[0:64, :], ata[0:64, :], rtA)
                nc.vector.tensor_mul(ao[64:128, :], atb[64:128, :], rtB)

        # ---------------- bias + out projection ----------------
        bias = big.tile([P, C], f32, name="bias")
        bl = const.tile([1, C], f32)
        nc.gpsimd.dma_start(bl, bout_h.unsqueeze(0))
        for hh in range(2):
            ps = ps_pool.tile([P, QB], f32, tag="ps")
            nc.tensor.matmul(ps, onesrow, bl[0:1, hh * QB : (hh + 1) * QB])
            nc.scalar.copy(bias[:, hh * QB : (hh + 1) * QB], ps)

        woT0 = w_chunk_T(wout_h, 0, wtv, "wtv")
        woT1 = w_chunk_T(wout_h, 4, wtv, "wtv")
        for i in range(TQ // P):
            ob = outp.tile([P, C], f32, tag="ob")
            for oh, woT in ((0, woT0), (1, woT1)):
                ps = ps_pool.tile([P, QB], f32, tag="ps")
                for g in range(CG):
                    nc.tensor.matmul(
                        ps,
                        aoT[g][:, i * P : (i + 1) * P],
                        woT[:, g, :],
                        start=(g == 0),
                        stop=(g == CG - 1),
                    )
                nc.vector.tensor_add(
                    ob[:, oh * QB : (oh + 1) * QB],
                    ps,
                    bias[:, oh * QB : (oh + 1) * QB],
                )
            nc.sync.dma_start(y_h[i * P : (i + 1) * P, :], ob)


def _run(in_maps, trace=False):
    from concourse.bass_utils import run_bass_kernel_spmd

    if "nc" not in _CACHE:
        _CACHE["nc"] = _build()
    nc = _CACHE["nc"]
    return run_bass_kernel_spmd(
        nc, in_maps, core_ids=list(range(8)), trace=trace
    )


def _make_in_maps(x, w_qkv, w_out, b_out):
    x = np.ascontiguousarray(np.asarray(x, dtype=np.float32))
    w_qkv = np.ascontiguousarray(np.asarray(w_qkv, dtype=np.float32))
    w_out = np.ascontiguousarray(np.asarray(w_out, dtype=np.float32))
    b_out = np.ascontiguousarray(np.asarray(b_out, dtype=np.float32))
    in_maps = []
    for c in range(8):
        b, h = divmod(c, 2)
        xb = x[b]
        if h:
            xb = np.ascontiguousarray(
                np.concatenate([xb[TQ:], xb[:TQ]], axis=0)
            )
        in_maps.append(
            {"x": xb, "w_qkv": w_qkv, "w_out": w_out, "b_out": b_out}
        )
    return in_maps


def _gather(results):
    y = np.empty((B, N, C), dtype=np.float32)
    for c in range(8):
        b, h = divmod(c, 2)
        y[b, h * TQ : (h + 1) * TQ, :] = results[c]["y"]
    return y


def kernel(x, w_qkv, w_out, b_out):
    res = _run(_make_in_maps(x, w_qkv, w_out, b_out), trace=False)
    return _gather(res.results)



# revision 9
# speedup vs baseline: 1.0093x; 1.0093x over previous
"""Multi-head attention (qkv proj + softmax attention + out proj) on 8 trn2 cores.

Sharding: zero-collective. Core c handles batch b=c//2 and query-half h=c%2
(1024 queries). The host passes x[b] ROTATED by h*1024 tokens so that each
core's queries are always local tokens 0..1023 while K/V still cover all 2048
tokens (key order is irrelevant to softmax). Host concatenates the 8 output
slabs [1024, 1024] into [4, 2048, 1024].

Per-core kernel (all on one NeuronCore, Tile-scheduled):
  1. PE-transpose x and the weights to contraction-major layout (fp32 in,
     bf16 out via the DVE PSUM-evacuation copy).
  2. QKV projection in bf16: QT/KT stored feature-major [d, t] so each
     128-row tile holds a PAIR of heads (2 x 64 d-rows); V stored token-major
     with interleaved ones columns (fused softmax-sum rows come out of the
     AV matmul for free).
  3. Attention per head-pair: QK^T into PSUM, softmax WITHOUT
     max-subtraction (scores are ~N(0,1); fp32 exp is safe), exp on ScalarE
     straight out of PSUM, AV+sums fused, fast approximate reciprocal on DVE.
  4. Out-projection in bf16 + bias, fp32 result.
"""

import numpy as np

B, N, C = 4, 2048, 1024
H, D = 16, 64
P = 128
CG = C // P            # 8 contraction groups
TG = N // P            # 16 key-token chunks
TQ = N // 2            # 1024 queries per core
QB = 512               # query block (psum bank)
NB = TQ // QB          # 2
NPAIR = H // 2         # 8 head pairs
SCALE = 1.0 / np.sqrt(D).astype(np.float32)
DEBUG_DUMPS = False

_CACHE = {}


def _build():
    import concourse.tile as tile
    from concourse import bacc, mybir

    f32 = mybir.dt.float32
    nc = bacc.Bacc(
        "TRN2", target_bir_lowering=False, debug=False, num_devices=8
    )
    x_h = nc.dram_tensor("x", [N, C], f32, kind="ExternalInput").ap()
    wqkv_h = nc.dram_tensor("w_qkv", [3 * C, C], f32, kind="ExternalInput").ap()
    wout_h = nc.dram_tensor("w_out", [C, C], f32, kind="ExternalInput").ap()
    bout_h = nc.dram_tensor("b_out", [C], f32, kind="ExternalInput").ap()
    y_h = nc.dram_tensor("y", [TQ, C], f32, kind="ExternalOutput").ap()

    with tile.TileContext(nc) as tc:
        _emit(tc, x_h, wqkv_h, wout_h, bout_h, y_h)
    nc.compile()
    return nc


def _emit(tc, x_h, wqkv_h, wout_h, bout_h, y_h):
    from contextlib import ExitStack

    from concourse import mybir
    from concourse.masks import make_identity

    f32 = mybir.dt.float32
    bf16 = mybir.dt.bfloat16
    AF = mybir.ActivationFunctionType
    nc = tc.nc

    with ExitStack() as ctx:
        # ---------------- pools ----------------
        const = ctx.enter_context(tc.tile_pool(name="const", bufs=1))
        big = ctx.enter_context(tc.tile_pool(name="big", bufs=1))
        land = ctx.enter_context(tc.tile_pool(name="land", bufs=3))
        cstp = ctx.enter_context(tc.tile_pool(name="cstp", bufs=7))
        ktp = ctx.enter_context(tc.tile_pool(name="ktp", bufs=2))
        qtp = ctx.enter_context(tc.tile_pool(name="qtp", bufs=2))
        wtq = ctx.enter_context(tc.tile_pool(name="wtq", bufs=4))
        wtv = ctx.enter_context(tc.tile_pool(name="wtv", bufs=2))
        ptp = ctx.enter_context(tc.tile_pool(name="ptp", bufs=13))
        recp = ctx.enter_context(tc.tile_pool(name="recp", bufs=2))
        outp = ctx.enter_context(tc.tile_pool(name="outp", bufs=2))
        ps_pool = ctx.enter_context(
            tc.tile_pool(name="ps_pool", bufs=2, space="PSUM")
        )
        st_ps = ctx.enter_context(
            tc.tile_pool(name="st_ps", bufs=2, space="PSUM")
        )
        at_ps = ctx.enter_context(
            tc.tile_pool(name="at_ps", bufs=2, space="PSUM")
        )

        # ---------------- constants ----------------
        ident = const.tile([P, P], bf16)
        make_identity(nc, ident)
        onesrow = const.tile([1, P], f32)
        nc.gpsimd.memset(onesrow, 1.0)

        # ---------------- persistent tensors ----------------
        # V layout per pair p: cols [192p,192p+64)=V_h2p, [192p+64,+128)=ones,
        # [192p+128,+192)=V_h2p+1. The AV stationary operand for head A is
        # cols [192p,192p+128) = [V_A | ones] (output rows 0:64 = attention
        # out, rows 64:128 = softmax sums, replicated); for head B it is cols
        # [192p+64,192p+192) = [ones | V_B] (sums on rows 0:64, attention out
        # on rows 64:128). The ones block is shared between the two heads.
        xT = [big.tile([P, N], bf16, name=f"xT{g}") for g in range(CG)]
        V = [big.tile([P, 192 * NPAIR], bf16, name=f"V{i}") for i in range(TG)]
        aoT = [big.tile([P, TQ], bf16, name=f"aoT{g}") for g in range(CG)]
        for i in range(TG):
            v3i = V[i].rearrange("p (q e) -> p q e", e=64)
            nc.vector.memset(v3i[:, 1 : 3 * NPAIR : 3], 1.0)

        # -------- load x (fp32), cast bf16, PE-transpose into xT --------
        def load_cast(src_ap, name, scale=None, on_scalar=False):
            t = land.tile([P, C], f32, tag="land", name=f"ld{name}")
            nc.sync.dma_start(t, src_ap)
            b = cstp.tile([P, C], bf16, tag="cst", name=f"cs{name}")
            if on_scalar:
                if scale is None:
                    nc.scalar.copy(b, t)
                else:
                    nc.scalar.mul(b, t, scale)
            elif scale is None:
                nc.vector.tensor_copy(b, t)
            else:
                nc.vector.tensor_scalar_mul(b, t, scale)
            return b

        # x casts go on the scalar engine: it is idle until the first exp,
        # which parallelizes the cast chain with the psum-evac copies on DVE.
        xb16 = []
        for i in range(TG):
            xb16.append(
                load_cast(x_h[i * P : (i + 1) * P, :], f"x{i}", on_scalar=True)
            )

        # [128,128] bf16 PE transposes batched into one psum bank, one copy.
        # Smaller first batches so the PE starts as soon as chunk 0 lands.
        i4 = 0
        for nb in (1, 1, 2, 4, 4, 4):
            for g in range(CG):
                psf = ps_pool.tile([P, QB], f32, tag="ps")
                ps = psf[:, :].bitcast(bf16)[:, 0 : nb * P]
                for k in range(nb):
                    nc.tensor.transpose(
                        ps[:, k * P : (k + 1) * P],
                        xb16[i4 + k][:, g * P : (g + 1) * P],
                        ident,
                    )
                nc.vector.tensor_copy(xT[g][:, (i4) * P : (i4 + nb) * P], ps)
            i4 += nb

        def w_chunk_T(src_h, row0, pool, tag):
            """Load+transpose 4 consecutive 128-row chunks of a [*, C] fp32
            weight into a [P, CG, 512] bf16 tile (contraction-major)."""
            wt = pool.tile([P, CG, 4 * P], bf16, tag=tag)
            wn = []
            for jj in range(4):
                wn.append(
                    load_cast(
                        src_h[(row0 + jj) * P : (row0 + jj + 1) * P, :],
                        f"w{row0}_{jj}",
                    )
                )
            for g in range(CG):
                psf = ps_pool.tile([P, QB], f32, tag="ps")
                ps = psf[:, :].bitcast(bf16)[:, 0 : 4 * P]
                for jj in range(4):
                    nc.tensor.transpose(
                        ps[:, jj * P : (jj + 1) * P],
                        wn[jj][:, g * P : (g + 1) * P],
                        ident,
                    )
                nc.vector.tensor_copy(wt[:, g, :], ps)
            return wt

        # ---------------- V projection ----------------
        for fh in range(2):
            wvT = w_chunk_T(wqkv_h, 16 + 4 * fh, wtv, "wtv")
            for i in range(TG):
                ps = ps_pool.tile([P, QB], f32, tag="ps")
                for g in range(CG):
                    nc.tensor.matmul(
                        ps,
                        xT[g][:, i * P : (i + 1) * P],
                        wvT[:, g, :],
                        start=(g == 0),
                        stop=(g == CG - 1),
                    )
                # psum cols = v features [512*fh, 512*(fh+1)) = heads
                # 8fh..8fh+7. Scatter per-head 64-col blocks into the
                # interleaved V layout: head h -> col 192*(h//2)+128*(h%2).
                ps3 = ps.rearrange("p (k e) -> p k e", e=64)
                v3 = V[i].rearrange("p (q e) -> p q e", e=64)
                b0 = 12 * fh
                nc.vector.tensor_copy(v3[:, b0 : b0 + 12 : 3], ps3[:, 0::2])
                nc.vector.tensor_copy(
                    v3[:, b0 + 2 : b0 + 12 : 3], ps3[:, 1::2]
                )

        # ---------------- per-pair: K proj, Q proj, attention ----------------
        bias = None
        for p in range(NPAIR):
            if p == NPAIR - 1:
                # Prep the out-projection weights/bias one pair early so the
                # tail isn't serialized behind their DMA+cast+transpose chain.
                bias = big.tile([P, C], f32, name="bias")
                bl = const.tile([1, C], f32)
                nc.gpsimd.dma_start(bl, bout_h.unsqueeze(0))
                for hh in range(2):
                    ps = ps_pool.tile([P, QB], f32, tag="ps")
                    nc.tensor.matmul(
                        ps, onesrow, bl[0:1, hh * QB : (hh + 1) * QB]
                    )
                    nc.scalar.copy(bias[:, hh * QB : (hh + 1) * QB], ps)
                woT0 = w_chunk_T(wout_h, 0, wtv, "wtv")
                woT1 = w_chunk_T(wout_h, 4, wtv, "wtv")

            # K chunk p: w_qkv rows [C + p*128, C + (p+1)*128)
            KT = ktp.tile([P, N], bf16, tag="KT")
            wt = wtq.tile([P, CG, P], bf16, tag="wtq")
            wn = load_cast(wqkv_h[(8 + p) * P : (9 + p) * P, :], f"wk{p}")
            for g4 in range(0, CG, 4):
                psf = ps_pool.tile([P, QB], f32, tag="ps")
                ps = psf[:, :].bitcast(bf16)[:, 0 : 4 * P]
                for k in range(4):
                    nc.tensor.transpose(
                        ps[:, k * P : (k + 1) * P],
                        wn[:, (g4 + k) * P : (g4 + k + 1) * P],
                        ident,
                    )
                nc.vector.tensor_copy(wt[:, g4 : g4 + 4, :], ps)
            for tbp in range(0, 4, 2):
                pss = [
                    ps_pool.tile([P, QB], f32, tag="ps", name=f"pjk{k}")
                    for k in range(2)
                ]
                for g in range(CG):
                    for k in range(2):
                        nc.tensor.matmul(
                            pss[k],
                            wt[:, g, :],
                            xT[g][:, (tbp + k) * QB : (tbp + k + 1) * QB],
                            start=(g == 0),
                            stop=(g == CG - 1),
                        )
                for k in range(2):
                    nc.vector.tensor_copy(
                        KT[:, (tbp + k) * QB : (tbp + k + 1) * QB], pss[k]
                    )

            # Q chunk p: w_qkv rows [p*128, (p+1)*128); queries are tokens
            # 0..TQ-1 of the (rotated) local x. Scale 1/sqrt(D) folded into
            # the psum evacuation copy.
            QT = qtp.tile([P, TQ], bf16, tag="QT")
            wt = wtq.tile([P, CG, P], bf16, tag="wtq")
            wn = load_cast(
                wqkv_h[p * P : (p + 1) * P, :], f"wq{p}", scale=float(SCALE)
            )
            for g4 in range(0, CG, 4):
                psf = ps_pool.tile([P, QB], f32, tag="ps")
                ps = psf[:, :].bitcast(bf16)[:, 0 : 4 * P]
                for k in range(4):
                    nc.tensor.transpose(
                        ps[:, k * P : (k + 1) * P],
                        wn[:, (g4 + k) * P : (g4 + k + 1) * P],
                        ident,
                    )
                nc.vector.tensor_copy(wt[:, g4 : g4 + 4, :], ps)
            pss = [
                ps_pool.tile([P, QB], f32, tag="ps", name=f"pjq{k}")
                for k in range(2)
            ]
            for g in range(CG):
                for k in range(2):
                    nc.tensor.matmul(
                        pss[k],
                        wt[:, g, :],
                        xT[g][:, k * QB : (k + 1) * QB],
                        start=(g == 0),
                        stop=(g == CG - 1),
                    )
            for k in range(2):
                nc.vector.tensor_copy(QT[:, k * QB : (k + 1) * QB], pss[k])

            # ---- attention for head pair p ----
            for tb in range(NB):
                qa = QT[0:64, tb * QB : (tb + 1) * QB]
                qb = QT[64:128, tb * QB : (tb + 1) * QB]
                pts = []
                for jj in range(TG // 2):
                    sta = st_ps.tile([P, 2 * QB], f32, tag="st", name="sta")
                    stb = st_ps.tile([P, 2 * QB], f32, tag="st", name="stb")
                    for k in range(2):
                        j = 2 * jj + k
                        nc.tensor.matmul(
                            sta[:, k * QB : (k + 1) * QB],
                            KT[0:64, j * P : (j + 1) * P],
                            qa,
                        )
                        nc.tensor.matmul(
                            stb[:, k * QB : (k + 1) * QB],
                            KT[64:128, j * P : (j + 1) * P],
                            qb,
                        )
                    pta = ptp.tile([P, 2 * QB], bf16, tag="pt", name="pta")
                    ptb = ptp.tile([P, 2 * QB], bf16, tag="pt", name="ptb")
                    nc.scalar.activation(pta, sta, AF.Exp)
                    nc.scalar.activation(ptb, stb, AF.Exp)
                    pts.append((pta, ptb))

                # AV with fused softmax-sums (see V layout comment).
                ata = at_ps.tile([P, QB], f32, tag="at", name="ata")
                atb = at_ps.tile([P, QB], f32, tag="at", name="atb")
                for j in range(TG):
                    jj, k = divmod(j, 2)
                    pta, ptb = pts[jj]
                    first, last = (j == 0), (j == TG - 1)
                    nc.tensor.matmul(
                        ata,
                        V[j][:, 192 * p : 192 * p + 128],
                        pta[:, k * QB : (k + 1) * QB],
                        start=first,
                        stop=last,
                    )
                    nc.tensor.matmul(
                        atb,
                        V[j][:, 192 * p + 64 : 192 * p + 192],
                        ptb[:, k * QB : (k + 1) * QB],
                        start=first,
                        stop=last,
                    )

                # normalize: out = at * (1/sum). reciprocal_approx_fast
                # (custom-DVE) requires base-partition-0 APs, so stage sumsA
                # down to a base-0 tile; mixed PSUM+SBUF operands may use
                # different base partitions, so the muls read the reciprocal
                # tiles directly.
                combA = recp.tile([64, QB], f32, tag="combA", bufs=1)
                nc.vector.tensor_copy(combA, ata[64:128, :])
                rtA = recp.tile([64, QB], f32, tag="rtA", bufs=1)
                nc.vector.reciprocal_approx_fast(rtA, combA)
                rtB = recp.tile([64, QB], f32, tag="rtB", bufs=1)
                nc.vector.reciprocal_approx_fast(rtB, atb[0:64, :])
                ao = aoT[p][:, tb * QB : (tb + 1) * QB]
                nc.vector.tensor_mul(ao[0:64, :], ata[0:64, :], rtA)
                nc.vector.tensor_mul(ao[64:128, :], atb[64:128, :], rtB)

        # ---------------- out projection ----------------
        for i in range(TQ // P):
            ob = outp.tile([P, C], f32, tag="ob")
            for oh, woT in ((0, woT0), (1, woT1)):
                ps = ps_pool.tile([P, QB], f32, tag="ps")
                for g in range(CG):
                    nc.tensor.matmul(
                        ps,
                        aoT[g][:, i * P : (i + 1) * P],
                        woT[:, g, :],
                        start=(g == 0),
                        stop=(g == CG - 1),
                    )
                nc.vector.tensor_add(
                    ob[:, oh * QB : (oh + 1) * QB],
                    ps,
                    bias[:, oh * QB : (oh + 1) * QB],
                )
            nc.sync.dma_start(y_h[i * P : (i + 1) * P, :], ob)


def _run(in_maps, trace=False):
    from concourse.bass_utils import run_bass_kernel_spmd

    if "nc" not in _CACHE:
        _CACHE["nc"] = _build()
    nc = _CACHE["nc"]
    return run_bass_kernel_spmd(
        nc, in_maps, core_ids=list(range(8)), trace=trace
    )


def _make_in_maps(x, w_qkv, w_out, b_out):
    x = np.ascontiguousarray(np.asarray(x, dtype=np.float32))
    w_qkv = np.ascontiguousarray(np.asarray(w_qkv, dtype=np.float32))
    w_out = np.ascontiguousarray(np.asarray(w_out, dtype=np.float32))
    b_out = np.ascontiguousarray(np.asarray(b_out, dtype=np.float32))
    in_maps = []
    for c in range(8):
        b, h = divmod(c, 2)
        xb = x[b]
        if h:
            xb = np.ascontiguousarray(
                np.concatenate([xb[TQ:], xb[:TQ]], axis=0)
            )
        in_maps.append(
            {"x": xb, "w_qkv": w_qkv, "w_out": w_out, "b_out": b_out}
        )
    return in_maps


def _gather(results):
    y = np.empty((B, N, C), dtype=np.float32)
    for c in range(8):
        b, h = divmod(c, 2)
        y[b, h * TQ : (h + 1) * TQ, :] = results[c]["y"]
    return y


def kernel(x, w_qkv, w_out, b_out):
    res = _run(_make_in_maps(x, w_qkv, w_out, b_out), trace=False)
    return _gather(res.results)



# revision 12
# speedup vs baseline: 1.0208x; 1.0113x over previous
"""Multi-head attention (qkv proj + softmax attention + out proj) on 8 trn2 cores.

Sharding: zero-collective. Core c handles batch b=c//2 and query-half h=c%2
(1024 queries). The host passes x[b] ROTATED by h*1024 tokens so that each
core's queries are always local tokens 0..1023 while K/V still cover all 2048
tokens (key order is irrelevant to softmax). Host concatenates the 8 output
slabs [1024, 1024] into [4, 2048, 1024].

Per-core kernel (all on one NeuronCore, Tile-scheduled):
  1. PE-transpose x and the weights to contraction-major layout (fp32 in,
     bf16 out via the DVE PSUM-evacuation copy).
  2. QKV projection in bf16: QT/KT stored feature-major [d, t] so each
     128-row tile holds a PAIR of heads (2 x 64 d-rows); V stored token-major
     with interleaved ones columns (fused softmax-sum rows come out of the
     AV matmul for free).
  3. Attention per head-pair: QK^T into PSUM, softmax WITHOUT
     max-subtraction (scores are ~N(0,1); fp32 exp is safe), exp on ScalarE
     straight out of PSUM, AV+sums fused, fast approximate reciprocal on DVE.
  4. Out-projection in bf16 + bias, fp32 result.
"""

import numpy as np

B, N, C = 4, 2048, 1024
H, D = 16, 64
P = 128
CG = C // P            # 8 contraction groups
TG = N // P            # 16 key-token chunks
TQ = N // 2            # 1024 queries per core
QB = 512               # query block (psum bank)
NB = TQ // QB          # 2
NPAIR = H // 2         # 8 head pairs
SCALE = 1.0 / np.sqrt(D).astype(np.float32)
DEBUG_DUMPS = False

_CACHE = {}


def _build():
    import concourse.tile as tile
    from concourse import bacc, mybir

    f32 = mybir.dt.float32
    nc = bacc.Bacc(
        "TRN2", target_bir_lowering=False, debug=False, num_devices=8
    )
    x_h = nc.dram_tensor("x", [N, C], f32, kind="ExternalInput").ap()
    wqkv_h = nc.dram_tensor("w_qkv", [3 * C, C], f32, kind="ExternalInput").ap()
    wout_h = nc.dram_tensor("w_out", [C, C], f32, kind="ExternalInput").ap()
    bout_h = nc.dram_tensor("b_out", [C], f32, kind="ExternalInput").ap()
    y_h = nc.dram_tensor("y", [TQ, C], f32, kind="ExternalOutput").ap()

    with tile.TileContext(nc) as tc:
        _emit(tc, x_h, wqkv_h, wout_h, bout_h, y_h)
    nc.compile()
    return nc


def _emit(tc, x_h, wqkv_h, wout_h, bout_h, y_h):
    from contextlib import ExitStack

    from concourse import mybir
    from concourse.masks import make_identity

    f32 = mybir.dt.float32
    bf16 = mybir.dt.bfloat16
    AF = mybir.ActivationFunctionType
    nc = tc.nc

    with ExitStack() as ctx:
        # ---------------- pools ----------------
        const = ctx.enter_context(tc.tile_pool(name="const", bufs=1))
        big = ctx.enter_context(tc.tile_pool(name="big", bufs=1))
        land = ctx.enter_context(tc.tile_pool(name="land", bufs=3))
        cstp = ctx.enter_context(tc.tile_pool(name="cstp", bufs=7))
        ktp = ctx.enter_context(tc.tile_pool(name="ktp", bufs=2))
        qtp = ctx.enter_context(tc.tile_pool(name="qtp", bufs=2))
        wtq = ctx.enter_context(tc.tile_pool(name="wtq", bufs=4))
        wtv = ctx.enter_context(tc.tile_pool(name="wtv", bufs=2))
        ptp = ctx.enter_context(tc.tile_pool(name="ptp", bufs=13))
        recp = ctx.enter_context(tc.tile_pool(name="recp", bufs=2))
        outp = ctx.enter_context(tc.tile_pool(name="outp", bufs=2))
        ps_pool = ctx.enter_context(
            tc.tile_pool(name="ps_pool", bufs=2, space="PSUM")
        )
        st_ps = ctx.enter_context(
            tc.tile_pool(name="st_ps", bufs=2, space="PSUM")
        )
        at_ps = ctx.enter_context(
            tc.tile_pool(name="at_ps", bufs=2, space="PSUM")
        )

        # ---------------- constants ----------------
        ident = const.tile([P, P], bf16)
        make_identity(nc, ident)
        onesrow = const.tile([1, P], f32)
        nc.gpsimd.memset(onesrow, 1.0)

        # ---------------- persistent tensors ----------------
        # V layout per pair p: cols [192p,192p+64)=V_h2p, [192p+64,+128)=ones,
        # [192p+128,+192)=V_h2p+1. The AV stationary operand for head A is
        # cols [192p,192p+128) = [V_A | ones] (output rows 0:64 = attention
        # out, rows 64:128 = softmax sums, replicated); for head B it is cols
        # [192p+64,192p+192) = [ones | V_B] (sums on rows 0:64, attention out
        # on rows 64:128). The ones block is shared between the two heads.
        xT = [big.tile([P, N], bf16, name=f"xT{g}") for g in range(CG)]
        V = [big.tile([P, 192 * NPAIR], bf16, name=f"V{i}") for i in range(TG)]
        aoT = [big.tile([P, TQ], bf16, name=f"aoT{g}") for g in range(CG)]
        for i in range(TG):
            v3i = V[i].rearrange("p (q e) -> p q e", e=64)
            nc.vector.memset(v3i[:, 1 : 3 * NPAIR : 3], 1.0)

        # -------- load x (fp32), cast bf16, PE-transpose into xT --------
        def load_cast(src_ap, name, scale=None, on_scalar=False):
            t = land.tile([P, C], f32, tag="land", name=f"ld{name}")
            nc.sync.dma_start(t, src_ap)
            b = cstp.tile([P, C], bf16, tag="cst", name=f"cs{name}")
            if on_scalar:
                if scale is None:
                    nc.scalar.copy(b, t)
                else:
                    nc.scalar.mul(b, t, scale)
            elif scale is None:
                nc.vector.tensor_copy(b, t)
            else:
                nc.vector.tensor_scalar_mul(b, t, scale)
            return b

        # x casts go on the scalar engine: it is idle until the first exp,
        # which parallelizes the cast chain with the psum-evac copies on DVE.
        xb16 = []
        for i in range(TG):
            xb16.append(
                load_cast(x_h[i * P : (i + 1) * P, :], f"x{i}", on_scalar=True)
            )

        # [128,128] bf16 PE transposes batched into one psum bank, one copy.
        # Smaller first batches so the PE starts as soon as chunk 0 lands.
        i4 = 0
        for nb in (1, 1, 2, 4, 4, 4):
            for g in range(CG):
                psf = ps_pool.tile([P, QB], f32, tag="ps")
                ps = psf[:, :].bitcast(bf16)[:, 0 : nb * P]
                for k in range(nb):
                    nc.tensor.transpose(
                        ps[:, k * P : (k + 1) * P],
                        xb16[i4 + k][:, g * P : (g + 1) * P],
                        ident,
                    )
                nc.vector.tensor_copy(xT[g][:, (i4) * P : (i4 + nb) * P], ps)
            i4 += nb

        def w_chunk_T(src_h, row0, pool, tag):
            """Load+transpose 4 consecutive 128-row chunks of a [*, C] fp32
            weight into a [P, CG, 512] bf16 tile (contraction-major)."""
            wt = pool.tile([P, CG, 4 * P], bf16, tag=tag)
            wn = []
            for jj in range(4):
                wn.append(
                    load_cast(
                        src_h[(row0 + jj) * P : (row0 + jj + 1) * P, :],
                        f"w{row0}_{jj}",
                    )
                )
            for g in range(CG):
                psf = ps_pool.tile([P, QB], f32, tag="ps")
                ps = psf[:, :].bitcast(bf16)[:, 0 : 4 * P]
                for jj in range(4):
                    nc.tensor.transpose(
                        ps[:, jj * P : (jj + 1) * P],
                        wn[jj][:, g * P : (g + 1) * P],
                        ident,
                    )
                nc.vector.tensor_copy(wt[:, g, :], ps)
            return wt

        # ---------------- V projection ----------------
        for fh in range(2):
            wvT = w_chunk_T(wqkv_h, 16 + 4 * fh, wtv, "wtv")
            for i in range(TG):
                ps = ps_pool.tile([P, QB], f32, tag="ps")
                for g in range(CG):
                    nc.tensor.matmul(
                        ps,
                        xT[g][:, i * P : (i + 1) * P],
                        wvT[:, g, :],
                        start=(g == 0),
                        stop=(g == CG - 1),
                    )
                # psum cols = v features [512*fh, 512*(fh+1)) = heads
                # 8fh..8fh+7. Scatter per-head 64-col blocks into the
                # interleaved V layout: head h -> col 192*(h//2)+128*(h%2).
                ps3 = ps.rearrange("p (k e) -> p k e", e=64)
                v3 = V[i].rearrange("p (q e) -> p q e", e=64)
                b0 = 12 * fh
                nc.vector.tensor_copy(v3[:, b0 : b0 + 12 : 3], ps3[:, 0::2])
                nc.vector.tensor_copy(
                    v3[:, b0 + 2 : b0 + 12 : 3], ps3[:, 1::2]
                )

        # ---------------- per-pair: K proj, Q proj, attention ----------------
        def prep_w(src_row0, name, scale=None):
            """DMA + cast + PE-transpose one 128-row w_qkv chunk into a
            contraction-major [P, CG, P] bf16 tile."""
            wt = wtq.tile([P, CG, P], bf16, tag="wtq")
            wn = load_cast(
                wqkv_h[src_row0 * P : (src_row0 + 1) * P, :], name, scale=scale
            )
            for g4 in range(0, CG, 4):
                psf = ps_pool.tile([P, QB], f32, tag="ps")
                ps = psf[:, :].bitcast(bf16)[:, 0 : 4 * P]
                for k in range(4):
                    nc.tensor.transpose(
                        ps[:, k * P : (k + 1) * P],
                        wn[:, (g4 + k) * P : (g4 + k + 1) * P],
                        ident,
                    )
                nc.vector.tensor_copy(wt[:, g4 : g4 + 4, :], ps)
            return wt

        def prep_pair(p):
            # K chunk p: w_qkv rows [C + p*128, C + (p+1)*128); Q chunk p:
            # rows [p*128, (p+1)*128) with 1/sqrt(D) folded into the cast.
            return (
                prep_w(8 + p, f"wk{p}"),
                prep_w(p, f"wq{p}", scale=float(SCALE)),
            )

        bias = None
        prep = prep_pair(0)
        for p in range(NPAIR):
            wtK, wtQ = prep
            KT = ktp.tile([P, N], bf16, tag="KT")
            for tbp in range(0, 4, 2):
                pss = [
                    ps_pool.tile([P, QB], f32, tag="ps", name=f"pjk{k}")
                    for k in range(2)
                ]
                for g in range(CG):
                    for k in range(2):
                        nc.tensor.matmul(
                            pss[k],
                            wtK[:, g, :],
                            xT[g][:, (tbp + k) * QB : (tbp + k + 1) * QB],
                            start=(g == 0),
                            stop=(g == CG - 1),
                        )
                for k in range(2):
                    nc.vector.tensor_copy(
                        KT[:, (tbp + k) * QB : (tbp + k + 1) * QB], pss[k]
                    )

            # Q proj: queries are tokens 0..TQ-1 of the (rotated) local x.
            QT = qtp.tile([P, TQ], bf16, tag="QT")
            pss = [
                ps_pool.tile([P, QB], f32, tag="ps", name=f"pjq{k}")
                for k in range(2)
            ]
            for g in range(CG):
                for k in range(2):
                    nc.tensor.matmul(
                        pss[k],
                        wtQ[:, g, :],
                        xT[g][:, k * QB : (k + 1) * QB],
                        start=(g == 0),
                        stop=(g == CG - 1),
                    )
            for k in range(2):
                nc.vector.tensor_copy(QT[:, k * QB : (k + 1) * QB], pss[k])

            # Software pipeline: emit the NEXT pair's weight DMA+cast+
            # transpose before this pair's attention, so those vector casts
            # and PE transposes are not queued behind the attention
            # normalization chain (engines execute in emission order).
            if p + 1 < NPAIR:
                prep = prep_pair(p + 1)
            if p == NPAIR - 2:
                # Out-projection weights/bias prepped a pair early too.
                bias = big.tile([P, C], f32, name="bias")
                bl = const.tile([1, C], f32)
                nc.gpsimd.dma_start(bl, bout_h.unsqueeze(0))
                for hh in range(2):
                    ps = ps_pool.tile([P, QB], f32, tag="ps")
                    nc.tensor.matmul(
                        ps, onesrow, bl[0:1, hh * QB : (hh + 1) * QB]
                    )
                    nc.scalar.copy(bias[:, hh * QB : (hh + 1) * QB], ps)
                woT0 = w_chunk_T(wout_h, 0, wtv, "wtv")
                woT1 = w_chunk_T(wout_h, 4, wtv, "wtv")

            # ---- attention for head pair p ----
            for tb in range(NB):
                qa = QT[0:64, tb * QB : (tb + 1) * QB]
                qb = QT[64:128, tb * QB : (tb + 1) * QB]
                pts = []
                for jj in range(TG // 2):
                    sta = st_ps.tile([P, 2 * QB], f32, tag="st", name="sta")
                    stb = st_ps.tile([P, 2 * QB], f32, tag="st", name="stb")
                    for k in range(2):
                        j = 2 * jj + k
                        nc.tensor.matmul(
                            sta[:, k * QB : (k + 1) * QB],
                            KT[0:64, j * P : (j + 1) * P],
                            qa,
                        )
                        nc.tensor.matmul(
                            stb[:, k * QB : (k + 1) * QB],
                            KT[64:128, j * P : (j + 1) * P],
                            qb,
                        )
                    pta = ptp.tile([P, 2 * QB], bf16, tag="pt", name="pta")
                    ptb = ptp.tile([P, 2 * QB], bf16, tag="pt", name="ptb")
                    nc.scalar.activation(pta, sta, AF.Exp)
                    nc.scalar.activation(ptb, stb, AF.Exp)
                    pts.append((pta, ptb))

                # AV with fused softmax-sums (see V layout comment).
                ata = at_ps.tile([P, QB], f32, tag="at", name="ata")
                atb = at_ps.tile([P, QB], f32, tag="at", name="atb")
                for j in range(TG):
                    jj, k = divmod(j, 2)
                    pta, ptb = pts[jj]
                    first, last = (j == 0), (j == TG - 1)
                    nc.tensor.matmul(
                        ata,
                        V[j][:, 192 * p : 192 * p + 128],
                        pta[:, k * QB : (k + 1) * QB],
                        start=first,
                        stop=last,
                    )
                    nc.tensor.matmul(
                        atb,
                        V[j][:, 192 * p + 64 : 192 * p + 192],
                        ptb[:, k * QB : (k + 1) * QB],
                        start=first,
                        stop=last,
                    )

                # normalize: out = at * (1/sum). reciprocal_approx_fast
                # (custom-DVE) requires base-partition-0 APs, so stage sumsA
                # down to a base-0 tile; mixed PSUM+SBUF operands may use
                # different base partitions, so the muls read the reciprocal
                # tiles directly.
                combA = recp.tile([64, QB], f32, tag="combA", bufs=1)
                nc.vector.tensor_copy(combA, ata[64:128, :])
                rtA = recp.tile([64, QB], f32, tag="rtA", bufs=1)
                nc.vector.reciprocal_approx_fast(rtA, combA)
                rtB = recp.tile([64, QB], f32, tag="rtB", bufs=1)
                nc.vector.reciprocal_approx_fast(rtB, atb[0:64, :])
                ao = aoT[p][:, tb * QB : (tb + 1) * QB]
                nc.vector.tensor_mul(ao[0:64, :], ata[0:64, :], rtA)
                nc.vector.tensor_mul(ao[64:128, :], atb[64:128, :], rtB)

        # ---------------- out projection ----------------
        for i in range(TQ // P):
            ob = outp.tile([P, C], f32, tag="ob")
            for oh, woT in ((0, woT0), (1, woT1)):
                ps = ps_pool.tile([P, QB], f32, tag="ps")
                for g in range(CG):
                    nc.tensor.matmul(
                        ps,
                        aoT[g][:, i * P : (i + 1) * P],
                        woT[:, g, :],
                        start=(g == 0),
                        stop=(g == CG - 1),
                    )
                nc.vector.tensor_add(
                    ob[:, oh * QB : (oh + 1) * QB],
                    ps,
                    bias[:, oh * QB : (oh + 1) * QB],
                )
            nc.sync.dma_start(y_h[i * P : (i + 1) * P, :], ob)


def _run(in_maps, trace=False):
    from concourse.bass_utils import run_bass_kernel_spmd

    if "nc" not in _CACHE:
        _CACHE["nc"] = _build()
    nc = _CACHE["nc"]
    return run_bass_kernel_spmd(
        nc, in_maps, core_ids=list(range(8)), trace=trace
    )


def _make_in_maps(x, w_qkv, w_out, b_out):
    x = np.ascontiguousarray(np.asarray(x, dtype=np.float32))
    w_qkv = np.ascontiguousarray(np.asarray(w_qkv, dtype=np.float32))
    w_out = np.ascontiguousarray(np.asarray(w_out, dtype=np.float32))
    b_out = np.ascontiguousarray(np.asarray(b_out, dtype=np.float32))
    in_maps = []
    for c in range(8):
        b, h = divmod(c, 2)
        xb = x[b]
        if h:
            xb = np.ascontiguousarray(
                np.concatenate([xb[TQ:], xb[:TQ]], axis=0)
            )
        in_maps.append(
            {"x": xb, "w_qkv": w_qkv, "w_out": w_out, "b_out": b_out}
        )
    return in_maps


def _gather(results):
    y = np.empty((B, N, C), dtype=np.float32)
    for c in range(8):
        b, h = divmod(c, 2)
        y[b, h * TQ : (h + 1) * TQ, :] = results[c]["y"]
    return y


def kernel(x, w_qkv, w_out, b_out):
    res = _run(_make_in_maps(x, w_qkv, w_out, b_out), trace=False)
    return _gather(res.results)



# revision 13
# speedup vs baseline: 1.0890x; 1.0668x over previous
"""Multi-head attention (qkv proj + softmax attention + out proj) on 8 trn2 cores.

Sharding: zero-collective. Core c handles batch b=c//2 and query-half h=c%2
(1024 queries). The host passes x[b] ROTATED by h*1024 tokens so that each
core's queries are always local tokens 0..1023 while K/V still cover all 2048
tokens (key order is irrelevant to softmax). Host concatenates the 8 output
slabs [1024, 1024] into [4, 2048, 1024].

Per-core kernel (all on one NeuronCore, Tile-scheduled):
  1. PE-transpose x and the weights to contraction-major layout (fp32 in,
     bf16 out via the DVE PSUM-evacuation copy).
  2. QKV projection in bf16: QT/KT stored feature-major [d, t] so each
     128-row tile holds a PAIR of heads (2 x 64 d-rows); V stored token-major
     with interleaved ones columns (fused softmax-sum rows come out of the
     AV matmul for free).
  3. Attention per head-pair: QK^T into PSUM, softmax WITHOUT
     max-subtraction (scores are ~N(0,1); fp32 exp is safe), exp on ScalarE
     straight out of PSUM, AV+sums fused, fast approximate reciprocal on DVE.
  4. Out-projection in bf16 + bias, fp32 result.
"""

import numpy as np

B, N, C = 4, 2048, 1024
H, D = 16, 64
P = 128
CG = C // P            # 8 contraction groups
TG = N // P            # 16 key-token chunks
TQ = N // 2            # 1024 queries per core
QB = 512               # query block (psum bank)
NB = TQ // QB          # 2
NPAIR = H // 2         # 8 head pairs
SCALE = 1.0 / np.sqrt(D).astype(np.float32)
DEBUG_DUMPS = False

_CACHE = {}


def _build():
    import concourse.tile as tile
    from concourse import bacc, mybir

    f32 = mybir.dt.float32
    nc = bacc.Bacc(
        "TRN2", target_bir_lowering=False, debug=False, num_devices=8
    )
    x_h = nc.dram_tensor("x", [N, C], f32, kind="ExternalInput").ap()
    wqkv_h = nc.dram_tensor("w_qkv", [3 * C, C], f32, kind="ExternalInput").ap()
    wout_h = nc.dram_tensor("w_out", [C, C], f32, kind="ExternalInput").ap()
    bout_h = nc.dram_tensor("b_out", [C], f32, kind="ExternalInput").ap()
    y_h = nc.dram_tensor("y", [TQ, C], f32, kind="ExternalOutput").ap()

    with tile.TileContext(nc) as tc:
        _emit(tc, x_h, wqkv_h, wout_h, bout_h, y_h)
    nc.compile()
    return nc


def _emit(tc, x_h, wqkv_h, wout_h, bout_h, y_h):
    from contextlib import ExitStack

    from concourse import mybir
    from concourse.masks import make_identity

    f32 = mybir.dt.float32
    bf16 = mybir.dt.bfloat16
    AF = mybir.ActivationFunctionType
    nc = tc.nc

    with ExitStack() as ctx:
        # ---------------- pools ----------------
        const = ctx.enter_context(tc.tile_pool(name="const", bufs=1))
        big = ctx.enter_context(tc.tile_pool(name="big", bufs=1))
        land = ctx.enter_context(tc.tile_pool(name="land", bufs=3))
        cstp = ctx.enter_context(tc.tile_pool(name="cstp", bufs=7))
        ktp = ctx.enter_context(tc.tile_pool(name="ktp", bufs=2))
        qtp = ctx.enter_context(tc.tile_pool(name="qtp", bufs=2))
        wtq = ctx.enter_context(tc.tile_pool(name="wtq", bufs=4))
        wtv = ctx.enter_context(tc.tile_pool(name="wtv", bufs=2))
        ptp = ctx.enter_context(tc.tile_pool(name="ptp", bufs=13))
        recp = ctx.enter_context(tc.tile_pool(name="recp", bufs=2))
        outp = ctx.enter_context(tc.tile_pool(name="outp", bufs=2))
        ps_pool = ctx.enter_context(
            tc.tile_pool(name="ps_pool", bufs=2, space="PSUM")
        )
        st_ps = ctx.enter_context(
            tc.tile_pool(name="st_ps", bufs=2, space="PSUM")
        )
        at_ps = ctx.enter_context(
            tc.tile_pool(name="at_ps", bufs=2, space="PSUM")
        )

        # ---------------- constants ----------------
        ident = const.tile([P, P], bf16)
        make_identity(nc, ident)
        onesrow = const.tile([1, P], f32)
        nc.gpsimd.memset(onesrow, 1.0)

        # ---------------- persistent tensors ----------------
        # V layout per pair p: cols [192p,192p+64)=V_h2p, [192p+64,+128)=ones,
        # [192p+128,+192)=V_h2p+1. The AV stationary operand for head A is
        # cols [192p,192p+128) = [V_A | ones] (output rows 0:64 = attention
        # out, rows 64:128 = softmax sums, replicated); for head B it is cols
        # [192p+64,192p+192) = [ones | V_B] (sums on rows 0:64, attention out
        # on rows 64:128). The ones block is shared between the two heads.
        xT = [big.tile([P, N], bf16, name=f"xT{g}") for g in range(CG)]
        V = [big.tile([P, 192 * NPAIR], bf16, name=f"V{i}") for i in range(TG)]
        aoT = [big.tile([P, TQ], bf16, name=f"aoT{g}") for g in range(CG)]
        for i in range(TG):
            v3i = V[i].rearrange("p (q e) -> p q e", e=64)
            nc.vector.memset(v3i[:, 1 : 3 * NPAIR : 3], 1.0)

        # -------- load x (fp32), cast bf16, PE-transpose into xT --------
        def load_cast(src_ap, name, scale=None, on_scalar=False):
            t = land.tile([P, C], f32, tag="land", name=f"ld{name}")
            nc.sync.dma_start(t, src_ap)
            b = cstp.tile([P, C], bf16, tag="cst", name=f"cs{name}")
            if on_scalar:
                if scale is None:
                    nc.scalar.copy(b, t)
                else:
                    nc.scalar.mul(b, t, scale)
            elif scale is None:
                nc.vector.tensor_copy(b, t)
            else:
                nc.vector.tensor_scalar_mul(b, t, scale)
            return b

        # x casts go on the scalar engine: it is idle until the first exp,
        # which parallelizes the cast chain with the psum-evac copies on DVE.
        xb16 = []
        for i in range(TG):
            xb16.append(
                load_cast(x_h[i * P : (i + 1) * P, :], f"x{i}", on_scalar=True)
            )

        # [128,128] bf16 PE transposes batched into one psum bank, one copy.
        # Smaller first batches so the PE starts as soon as chunk 0 lands.
        i4 = 0
        for nb in (1, 1, 2, 4, 4, 4):
            for g in range(CG):
                psf = ps_pool.tile([P, QB], f32, tag="ps")
                ps = psf[:, :].bitcast(bf16)[:, 0 : nb * P]
                for k in range(nb):
                    nc.tensor.transpose(
                        ps[:, k * P : (k + 1) * P],
                        xb16[i4 + k][:, g * P : (g + 1) * P],
                        ident,
                    )
                nc.vector.tensor_copy(xT[g][:, (i4) * P : (i4 + nb) * P], ps)
            i4 += nb

        def w_chunk_T(src_h, row0, pool, tag):
            """Load+transpose 4 consecutive 128-row chunks of a [*, C] fp32
            weight into a [P, CG, 512] bf16 tile (contraction-major)."""
            wt = pool.tile([P, CG, 4 * P], bf16, tag=tag)
            wn = []
            for jj in range(4):
                wn.append(
                    load_cast(
                        src_h[(row0 + jj) * P : (row0 + jj + 1) * P, :],
                        f"w{row0}_{jj}",
                    )
                )
            for g in range(CG):
                psf = ps_pool.tile([P, QB], f32, tag="ps")
                ps = psf[:, :].bitcast(bf16)[:, 0 : 4 * P]
                for jj in range(4):
                    nc.tensor.transpose(
                        ps[:, jj * P : (jj + 1) * P],
                        wn[jj][:, g * P : (g + 1) * P],
                        ident,
                    )
                nc.vector.tensor_copy(wt[:, g, :], ps)
            return wt

        # ---------------- V projection ----------------
        for fh in range(2):
            wvT = w_chunk_T(wqkv_h, 16 + 4 * fh, wtv, "wtv")
            for i in range(TG):
                ps = ps_pool.tile([P, QB], f32, tag="ps")
                for g in range(CG):
                    nc.tensor.matmul(
                        ps,
                        xT[g][:, i * P : (i + 1) * P],
                        wvT[:, g, :],
                        start=(g == 0),
                        stop=(g == CG - 1),
                    )
                # psum cols = v features [512*fh, 512*(fh+1)) = heads
                # 8fh..8fh+7. Scatter per-head 64-col blocks into the
                # interleaved V layout: head h -> col 192*(h//2)+128*(h%2).
                ps3 = ps.rearrange("p (k e) -> p k e", e=64)
                v3 = V[i].rearrange("p (q e) -> p q e", e=64)
                b0 = 12 * fh
                nc.vector.tensor_copy(v3[:, b0 : b0 + 12 : 3], ps3[:, 0::2])
                nc.vector.tensor_copy(
                    v3[:, b0 + 2 : b0 + 12 : 3], ps3[:, 1::2]
                )

        # ---------------- per-pair: K proj, Q proj, attention ----------------
        def prep_w(src_row0, name, scale=None):
            """DMA + cast + PE-transpose one 128-row w_qkv chunk into a
            contraction-major [P, CG, P] bf16 tile."""
            wt = wtq.tile([P, CG, P], bf16, tag="wtq")
            wn = load_cast(
                wqkv_h[src_row0 * P : (src_row0 + 1) * P, :], name, scale=scale
            )
            for g4 in range(0, CG, 4):
                psf = ps_pool.tile([P, QB], f32, tag="ps")
                ps = psf[:, :].bitcast(bf16)[:, 0 : 4 * P]
                for k in range(4):
                    nc.tensor.transpose(
                        ps[:, k * P : (k + 1) * P],
                        wn[:, (g4 + k) * P : (g4 + k + 1) * P],
                        ident,
                    )
                nc.vector.tensor_copy(wt[:, g4 : g4 + 4, :], ps)
            return wt

        def prep_pair(p):
            # K chunk p: w_qkv rows [C + p*128, C + (p+1)*128); Q chunk p:
            # rows [p*128, (p+1)*128) with 1/sqrt(D) folded into the cast.
            return (
                prep_w(8 + p, f"wk{p}"),
                prep_w(p, f"wq{p}", scale=float(SCALE)),
            )

        def proj_pair(wts):
            """K proj (full 2048 tokens) + Q proj (local TQ) for one pair."""
            wtK, wtQ = wts
            KT = ktp.tile([P, N], bf16, tag="KT")
            for tbp in range(0, 4, 2):
                pss = [
                    ps_pool.tile([P, QB], f32, tag="ps", name=f"pjk{k}")
                    for k in range(2)
                ]
                for g in range(CG):
                    for k in range(2):
                        nc.tensor.matmul(
                            pss[k],
                            wtK[:, g, :],
                            xT[g][:, (tbp + k) * QB : (tbp + k + 1) * QB],
                            start=(g == 0),
                            stop=(g == CG - 1),
                        )
                for k in range(2):
                    nc.vector.tensor_copy(
                        KT[:, (tbp + k) * QB : (tbp + k + 1) * QB], pss[k]
                    )

            QT = qtp.tile([P, TQ], bf16, tag="QT")
            pss = [
                ps_pool.tile([P, QB], f32, tag="ps", name=f"pjq{k}")
                for k in range(2)
            ]
            for g in range(CG):
                for k in range(2):
                    nc.tensor.matmul(
                        pss[k],
                        wtQ[:, g, :],
                        xT[g][:, k * QB : (k + 1) * QB],
                        start=(g == 0),
                        stop=(g == CG - 1),
                    )
            for k in range(2):
                nc.vector.tensor_copy(QT[:, k * QB : (k + 1) * QB], pss[k])
            return KT, QT

        # Software pipeline one pair ahead: pair p+1's weight prep AND K/Q
        # projection (matmuls + psum-evac casts) are emitted BEFORE pair p's
        # attention. Engines execute in emission order, so this keeps the
        # next pair's vector evacs ahead of the serial recip/mul
        # normalization chain that ends each attention block.
        bias = None
        cur = proj_pair(prep_pair(0))
        for p in range(NPAIR):
            KT, QT = cur
            if p + 1 < NPAIR:
                cur = proj_pair(prep_pair(p + 1))
            if p == NPAIR - 2:
                # Out-projection weights/bias prepped a pair early too.
                bias = big.tile([P, C], f32, name="bias")
                bl = const.tile([1, C], f32)
                nc.gpsimd.dma_start(bl, bout_h.unsqueeze(0))
                for hh in range(2):
                    ps = ps_pool.tile([P, QB], f32, tag="ps")
                    nc.tensor.matmul(
                        ps, onesrow, bl[0:1, hh * QB : (hh + 1) * QB]
                    )
                    nc.scalar.copy(bias[:, hh * QB : (hh + 1) * QB], ps)
                woT0 = w_chunk_T(wout_h, 0, wtv, "wtv")
                woT1 = w_chunk_T(wout_h, 4, wtv, "wtv")

            # ---- attention for head pair p ----
            for tb in range(NB):
                qa = QT[0:64, tb * QB : (tb + 1) * QB]
                qb = QT[64:128, tb * QB : (tb + 1) * QB]
                pts = []
                for jj in range(TG // 2):
                    sta = st_ps.tile([P, 2 * QB], f32, tag="st", name="sta")
                    stb = st_ps.tile([P, 2 * QB], f32, tag="st", name="stb")
                    for k in range(2):
                        j = 2 * jj + k
                        nc.tensor.matmul(
                            sta[:, k * QB : (k + 1) * QB],
                            KT[0:64, j * P : (j + 1) * P],
                            qa,
                        )
                        nc.tensor.matmul(
                            stb[:, k * QB : (k + 1) * QB],
                            KT[64:128, j * P : (j + 1) * P],
                            qb,
                        )
                    pta = ptp.tile([P, 2 * QB], bf16, tag="pt", name="pta")
                    ptb = ptp.tile([P, 2 * QB], bf16, tag="pt", name="ptb")
                    nc.scalar.activation(pta, sta, AF.Exp)
                    nc.scalar.activation(ptb, stb, AF.Exp)
                    pts.append((pta, ptb))

                # AV with fused softmax-sums (see V layout comment).
                ata = at_ps.tile([P, QB], f32, tag="at", name="ata")
                atb = at_ps.tile([P, QB], f32, tag="at", name="atb")
                for j in range(TG):
                    jj, k = divmod(j, 2)
                    pta, ptb = pts[jj]
                    first, last = (j == 0), (j == TG - 1)
                    nc.tensor.matmul(
                        ata,
                        V[j][:, 192 * p : 192 * p + 128],
                        pta[:, k * QB : (k + 1) * QB],
                        start=first,
                        stop=last,
                    )
                    nc.tensor.matmul(
                        atb,
                        V[j][:, 192 * p + 64 : 192 * p + 192],
                        ptb[:, k * QB : (k + 1) * QB],
                        start=first,
                        stop=last,
                    )

                # normalize: out = at * (1/sum). reciprocal_approx_fast
                # (custom-DVE) requires base-partition-0 APs, so stage sumsA
                # down to a base-0 tile; mixed PSUM+SBUF operands may use
                # different base partitions, so the muls read the reciprocal
                # tiles directly.
                combA = recp.tile([64, QB], f32, tag="combA", bufs=1)
                nc.vector.tensor_copy(combA, ata[64:128, :])
                rtA = recp.tile([64, QB], f32, tag="rtA", bufs=1)
                nc.vector.reciprocal_approx_fast(rtA, combA)
                rtB = recp.tile([64, QB], f32, tag="rtB", bufs=1)
                nc.vector.reciprocal_approx_fast(rtB, atb[0:64, :])
                ao = aoT[p][:, tb * QB : (tb + 1) * QB]
                nc.vector.tensor_mul(ao[0:64, :], ata[0:64, :], rtA)
                nc.vector.tensor_mul(ao[64:128, :], atb[64:128, :], rtB)

        # ---------------- out projection ----------------
        for i in range(TQ // P):
            ob = outp.tile([P, C], f32, tag="ob")
            for oh, woT in ((0, woT0), (1, woT1)):
                ps = ps_pool.tile([P, QB], f32, tag="ps")
                for g in range(CG):
                    nc.tensor.matmul(
                        ps,
                        aoT[g][:, i * P : (i + 1) * P],
                        woT[:, g, :],
                        start=(g == 0),
                        stop=(g == CG - 1),
                    )
                nc.vector.tensor_add(
                    ob[:, oh * QB : (oh + 1) * QB],
                    ps,
                    bias[:, oh * QB : (oh + 1) * QB],
                )
            nc.sync.dma_start(y_h[i * P : (i + 1) * P, :], ob)


def _run(in_maps, trace=False):
    from concourse.bass_utils import run_bass_kernel_spmd

    if "nc" not in _CACHE:
        _CACHE["nc"] = _build()
    nc = _CACHE["nc"]
    return run_bass_kernel_spmd(
        nc, in_maps, core_ids=list(range(8)), trace=trace
    )


def _make_in_maps(x, w_qkv, w_out, b_out):
    x = np.ascontiguousarray(np.asarray(x, dtype=np.float32))
    w_qkv = np.ascontiguousarray(np.asarray(w_qkv, dtype=np.float32))
    w_out = np.ascontiguousarray(np.asarray(w_out, dtype=np.float32))
    b_out = np.ascontiguousarray(np.asarray(b_out, dtype=np.float32))
    in_maps = []
    for c in range(8):
        b, h = divmod(c, 2)
        xb = x[b]
        if h:
            xb = np.ascontiguousarray(
                np.concatenate([xb[TQ:], xb[:TQ]], axis=0)
            )
        in_maps.append(
            {"x": xb, "w_qkv": w_qkv, "w_out": w_out, "b_out": b_out}
        )
    return in_maps


def _gather(results):
    y = np.empty((B, N, C), dtype=np.float32)
    for c in range(8):
        b, h = divmod(c, 2)
        y[b, h * TQ : (h + 1) * TQ, :] = results[c]["y"]
    return y


def kernel(x, w_qkv, w_out, b_out):
    res = _run(_make_in_maps(x, w_qkv, w_out, b_out), trace=False)
    return _gather(res.results)

